# revision 40
# baseline (speedup 1.0000x reference)
"""Biquad lowpass filter (torchaudio lowpass_biquad, SR=24000, cutoff=8000, Q=0.707)
over wav [64, 480000], data-parallel across 8 TRN2 NeuronCores.

The biquad's poles have |z| = sqrt(a2) ~= 0.49, so its impulse response decays
below float32 resolution within ~48 samples. The IIR is therefore numerically
identical to a 64-tap causal FIR, evaluated on the TensorEngine.

Layout per core: 8 rows x 480000 samples = 128 chunks (8 rows x 16) of 30000
samples, one chunk per partition (all 16 DMA engines engaged). Time is cut
into 120-sample slices; each slice is PE-transposed so within-slice time sits
on partitions, then becomes the stationary operand of a float32r matmul
against banded coefficient matrices [HB | HA]: one matmul per slice writes
that slice's FIR output plus its tail contribution to the next slice,
staggered-accumulated in a single PSUM bank (a full-width start=True carry
matmul [HA | 0] defines every element's first write). The accumulated bank is
y in natural layout -> one copy -> contiguous DMA out. Input DMAs ride the
sync HWDGE ring, output DMAs the scalar ring, 2400 samples/partition per
transfer (1.23 MB), quad-buffered. Measured ~95 us on hardware vs a ~85 us
pure-DMA floor for the 30.7 MB/core of HBM traffic.
"""

import sys

sys.path.insert(0, "/opt/trn_rl_repo")

import numpy as np

import concourse.mybir as mybir
import concourse.tile as tile
from concourse import bacc
from concourse.bass_utils import run_bass_kernel_spmd

f32 = mybir.dt.float32
f32r = mybir.dt.float32r

# ---- problem constants ----------------------------------------------------
SR = 24000
CUTOFF = 8000.0
Q = 0.707

B_FULL, T = 64, 480000
N_CORES = 8
R = B_FULL // N_CORES          # rows per core
NCH = 16                       # chunks per row
P = R * NCH                    # 128 partitions (one chunk each)
L = T // NCH                   # 30000 samples per chunk
LS = 120                       # slice length
NSL = L // LS                  # 250 slices per chunk
S = 4                          # slices per matmul block
G = 5                          # matmul blocks per DMA transfer
D = 64                         # FIR taps kept (h[48] ~ 1e-15 already)
MMW = 256                      # matmul width (>=256 keeps float32r at 1 cyc/col)

MM_DT = f32r                   # conv matmul dtype
TP_DT = f32r                   # transpose dtype


def _filter_mats():
    w0 = 2.0 * np.pi * CUTOFF / SR
    alpha = np.sin(w0) / (2.0 * Q)
    cos_w0 = np.cos(w0)
    b0 = (1.0 - cos_w0) / 2.0
    b1 = 1.0 - cos_w0
    b2 = b0
    a0 = 1.0 + alpha
    a1 = -2.0 * cos_w0
    a2 = 1.0 - alpha
    b0, b1, b2, a1, a2 = (np.float32(b0 / a0), np.float32(b1 / a0),
                          np.float32(b2 / a0), np.float32(a1 / a0),
                          np.float32(a2 / a0))
    # impulse response in float64 using the float32-rounded coefficients
    h = np.zeros(D, dtype=np.float64)
    x1 = x2 = y1 = y2 = 0.0
    for t in range(D):
        x = 1.0 if t == 0 else 0.0
        y = (float(b0) * x + float(b1) * x1 + float(b2) * x2
             - float(a1) * y1 - float(a2) * y2)
        h[t] = y
        x2, x1 = x1, x
        y2, y1 = y1, y
    HB = np.zeros((LS, LS), dtype=np.float32)
    HA = np.zeros((LS, LS), dtype=np.float32)
    for k in range(LS):
        for i in range(LS):
            d = i - k
            if 0 <= d < D:
                HB[k, i] = h[d]
            d2 = LS + i - k
            if 0 < d2 < D:
                HA[k, i] = h[d2]
    pad = MMW - 2 * LS
    Zp = np.zeros((LS, pad), dtype=np.float32)
    Z = np.zeros((LS, LS), dtype=np.float32)
    HBA = np.concatenate([HB, HA, Zp], axis=1)            # [LS, MMW]
    ZHB = np.concatenate([Z, HB, Zp], axis=1)             # [LS, MMW]
    HAZ = np.concatenate(                                  # [LS, 512]
        [HA, np.zeros((LS, 512 - LS), dtype=np.float32)], axis=1)
    # one packed constant block: [128, 2*MMW + 512 + 128]
    def pad128(m):
        return np.concatenate(
            [m, np.zeros((128 - m.shape[0], m.shape[1]), np.float32)], axis=0)
    blk = np.concatenate(
        [pad128(HBA), pad128(ZHB), pad128(HAZ), np.eye(128, dtype=np.float32)],
        axis=1)
    return blk


def _filter_mats_bf16():
    import ml_dtypes
    blk = _filter_mats()          # f32 [128, 2*MMW + 512 + 128]
    HBA = blk[:, 0: MMW][:, : 2 * LS]                 # [128, 240]
    ZHB = blk[:, MMW: 2 * MMW][:, : 2 * LS]
    HAZ = blk[:, 2 * MMW: 2 * MMW + 512][:, : 4 * LS]  # [128, 480]
    ident = blk[:, 2 * MMW + 512:]
    b = np.concatenate([HBA, ZHB, HAZ, ident], axis=1)  # [128, 1088]
    return b.astype(ml_dtypes.bfloat16)


def _build_bf16():
    import ml_dtypes
    bf16 = mybir.dt.bfloat16
    W = 2 * LS                       # matmul width (bf16: 1 cyc/col at any width)
    CONST_np = _filter_mats_bf16()
    nc = bacc.Bacc("TRN2", target_bir_lowering=False)

    wav = nc.dram_tensor("wav", [R, T], f32, kind="ExternalInput")
    out = nc.dram_tensor("out", [R, T], f32, kind="ExternalOutput")
    const_d = nc.inline_tensor(CONST_np, name="constblk16")

    wav_ch = wav[:, :].rearrange("r (c l) -> (r c) l", c=NCH)   # [128, 30000]
    out_ch = out[:, :].rearrange("r (c l) -> (r c) l", c=NCH)

    subs = []
    done = 0
    while done < NSL:
        s = min(S, NSL - done)
        subs.append((done, s))
        done += s
    groups = [subs[i: i + G] for i in range(0, len(subs), G)]

    with tile.TileContext(nc) as tc:
        with (
            tc.tile_pool(name="const", bufs=1) as cpool,
            tc.tile_pool(name="io", bufs=5) as iopool,
            tc.tile_pool(name="work", bufs=6) as wpool,
            tc.tile_pool(name="psum", bufs=4, space="PSUM") as ppool,
        ):
            cblk = cpool.tile([128, 2 * W + 4 * LS + 128], bf16)
            nc.sync.dma_start(cblk[:], const_d[:, :])
            hBA = cblk[:LS, 0: W]
            hZB = cblk[:LS, W: 2 * W]
            hAZ = cblk[:LS, 2 * W: 2 * W + 4 * LS]
            ident = cblk[:, 2 * W + 4 * LS:]

            c0 = cpool.tile([P, LS], bf16)
            nc.gpsimd.memset(c0[:], 0.0)
            for r in range(R):
                nc.gpsimd.dma_start(
                    c0[r * NCH + 1: r * NCH + NCH, :],
                    wav_ch[r * NCH: r * NCH + NCH - 1, L - LS: L],
                )
            pc0 = ppool.tile([LS, P], bf16, tag="pt")
            nc.tensor.transpose(pc0[:], c0[:], ident)
            c0T = cpool.tile([LS, P], bf16)
            nc.scalar.copy(c0T[:], pc0[:, :])

            prev_slab = None
            prev_s = None
            for gi, grp in enumerate(groups):
                gbase = grp[0][0] * LS
                gw = sum(s for (_, s) in grp) * LS

                xin = iopool.tile([P, G * S * LS], bf16, tag="xin")
                nc.gpsimd.dma_start(xin[:, :gw], wav_ch[:, gbase: gbase + gw])
                yout = iopool.tile([P, G * S * LS], f32, tag="yout")

                for (sl0, s) in grp:
                    off = sl0 * LS - gbase
                    w = s * LS

                    pt = ppool.tile([LS, S * P], bf16, tag="pt")
                    for j in range(s):
                        nc.tensor.transpose(
                            pt[:, j * P: (j + 1) * P],
                            xin[:, off + j * LS: off + (j + 1) * LS],
                            ident,
                        )

                    slab = wpool.tile([LS, S * P], bf16, tag="slab")
                    nc.scalar.copy(slab[:, : s * P], pt[:, : s * P])
                    carry = (c0T[:, :] if prev_slab is None
                             else prev_slab[:, (prev_s - 1) * P: prev_s * P])

                    py = ppool.tile([P, S * LS], f32, tag="py")
                    cw = s * LS
                    nc.tensor.matmul(
                        py[:, : cw], carry, hAZ[:, : cw],
                        start=True, stop=False,
                    )
                    for j in range(s):
                        if j < s - 1:
                            o, hmat = j * LS, hBA
                        else:
                            o, hmat = (s - 2) * LS, hZB
                        nc.tensor.matmul(
                            py[:, o: o + W],
                            slab[:, j * P: (j + 1) * P],
                            hmat,
                            start=False, stop=(j == s - 1),
                        )

                    nc.vector.tensor_copy(yout[:, off: off + w], py[:, :w])

                    prev_slab = slab
                    prev_s = s

                nc.scalar.dma_start(out_ch[:, gbase: gbase + gw], yout[:, :gw])

    nc.finalize()
    return nc


BF16_IN = False
BF16_TRANSIT = True


def _build():
    if BF16_IN:
        return _build_bf16()
    CONST_np = _filter_mats()
    nc = bacc.Bacc("TRN2", target_bir_lowering=False)

    wav = nc.dram_tensor("wav", [R, T], f32, kind="ExternalInput")
    out = nc.dram_tensor("out", [R, T], f32, kind="ExternalOutput")
    const_d = nc.inline_tensor(CONST_np, name="constblk")

    wav_ch = wav[:, :].rearrange("r (c l) -> (r c) l", c=NCH)   # [128, 30000]
    out_ch = out[:, :].rearrange("r (c l) -> (r c) l", c=NCH)

    # sub-iterations of up to S slices, grouped G sub-iters per DMA
    subs = []
    done = 0
    while done < NSL:
        s = min(S, NSL - done)
        subs.append((done, s))
        done += s
    groups = [subs[i: i + G] for i in range(0, len(subs), G)]

    with tile.TileContext(nc) as tc:
        with (
            tc.tile_pool(name="const", bufs=1) as cpool,
            tc.tile_pool(name="io", bufs=5) as iopool,
            tc.tile_pool(name="work", bufs=6) as wpool,
            tc.tile_pool(name="psum", bufs=4, space="PSUM") as ppool,
        ):
            cblk = cpool.tile([128, 2 * MMW + 512 + 128], f32)
            nc.sync.dma_start(cblk[:].bitcast(MM_DT), const_d[:, :].bitcast(MM_DT))
            hBA = cblk[:LS, 0: MMW]
            hZB = cblk[:LS, MMW: 2 * MMW]
            hAZ = cblk[:LS, 2 * MMW: 2 * MMW + 512]
            ident = cblk[:, 2 * MMW + 512:]

            # initial carry: the LS samples preceding each chunk (zeros for
            # row-initial chunks), transposed into slab-slice layout.
            c0 = cpool.tile([P, LS], f32)
            nc.gpsimd.memset(c0[:], 0.0)
            for r in range(R):
                nc.gpsimd.dma_start(
                    c0[r * NCH + 1: r * NCH + NCH, :],
                    wav_ch[r * NCH: r * NCH + NCH - 1, L - LS: L],
                )
            pc0 = ppool.tile([LS, P], f32, tag="pt")
            nc.tensor.transpose(pc0[:], c0[:], ident)
            c0T = cpool.tile([LS, P], f32)
            nc.scalar.copy(c0T[:].bitcast(MM_DT), pc0[:, :])

            prev_slab = None
            prev_s = None
            for gi, grp in enumerate(groups):
                gbase = grp[0][0] * LS
                gw = sum(s for (_, s) in grp) * LS

                xin = iopool.tile([P, G * S * LS], f32, tag="xin")
                if BF16_TRANSIT:
                    # SWDGE casts f32->bf16 inside the DMA engines: halves the
                    # SBUF-port traffic of the input stream; DVE upcasts back.
                    xin16 = iopool.tile([P, G * S * LS], mybir.dt.bfloat16,
                                        tag="xin16")
                    nc.gpsimd.dma_start(xin16[:, :gw],
                                        wav_ch[:, gbase: gbase + gw])
                    nc.vector.tensor_copy(xin[:, :gw].bitcast(TP_DT),
                                          xin16[:, :gw])
                else:
                    nc.sync.dma_start(
                        xin[:, :gw].bitcast(TP_DT),
                        wav_ch[:, gbase: gbase + gw].bitcast(TP_DT),
                    )
                yout = iopool.tile([P, G * S * LS], f32, tag="yout")

                for (sl0, s) in grp:
                    off = sl0 * LS - gbase      # sample offset within group
                    w = s * LS

                    # transpose s slices of [P, LS] -> [LS, P] into PSUM
                    pt = ppool.tile([LS, S * P], f32, tag="pt")
                    for j in range(s):
                        nc.tensor.transpose(
                            pt[:, j * P: (j + 1) * P].bitcast(TP_DT),
                            xin[:, off + j * LS: off + (j + 1) * LS].bitcast(TP_DT),
                            ident.bitcast(TP_DT),
                        )

                    slab = wpool.tile([LS, S * P], f32, tag="slab")
                    nc.scalar.copy(slab[:, : s * P].bitcast(MM_DT), pt[:, : s * P])
                    carry = (c0T[:, :] if prev_slab is None
                             else prev_slab[:, (prev_s - 1) * P: prev_s * P])

                    # staggered-accumulation FIR, y in natural layout:
                    #   carry matmul (start=True, [HA|0...]) covers every column
                    #   later matmuls touch; matmul j accumulates [HB|HA|0] at
                    #   j*LS; the last one uses [0|HB|0] shifted back to stay
                    #   inside the bank.
                    py = ppool.tile([P, S * LS + (MMW - 2 * LS)], f32, tag="py")
                    cw = min(512, (s - 2) * LS + MMW)
                    nc.tensor.matmul(
                        py[:, : cw],
                        carry.bitcast(MM_DT),
                        hAZ[:, : cw].bitcast(MM_DT),
                        start=True, stop=False,
                    )
                    for j in range(s):
                        if j < s - 1:
                            o, hmat = j * LS, hBA
                        else:
                            o, hmat = (s - 2) * LS, hZB
                        nc.tensor.matmul(
                            py[:, o: o + MMW],
                            slab[:, j * P: (j + 1) * P].bitcast(MM_DT),
                            hmat.bitcast(MM_DT),
                            start=False, stop=(j == s - 1),
                        )

                    nc.vector.tensor_copy(yout[:, off: off + w], py[:, :w])

                    prev_slab = slab
                    prev_s = s

                nc.scalar.dma_start(out_ch[:, gbase: gbase + gw], yout[:, :gw])

    nc.finalize()
    return nc


_NC_CACHE = None


def _get_nc():
    global _NC_CACHE
    if _NC_CACHE is None:
        _NC_CACHE = _build()
    return _NC_CACHE


def _run(wav_full: np.ndarray, trace: bool = False):
    nc = _get_nc()
    wav_full = np.ascontiguousarray(wav_full, dtype=np.float32)
    in_maps = [
        {"wav": wav_full[i * R: (i + 1) * R]} for i in range(N_CORES)
    ]
    res = run_bass_kernel_spmd(
        nc, in_maps, core_ids=list(range(N_CORES)), trace=trace
    )
    out = np.concatenate([res.results[i]["out"] for i in range(N_CORES)], axis=0)
    return out, res


def kernel(wav: np.ndarray) -> np.ndarray:
    out, _ = _run(np.asarray(wav))
    return out


# revision 41
# speedup vs baseline: 1.0388x; 1.0388x over previous
"""Biquad lowpass filter (torchaudio lowpass_biquad, SR=24000, cutoff=8000, Q=0.707)
over wav [64, 480000], data-parallel across 8 TRN2 NeuronCores.

The biquad's poles have |z| = sqrt(a2) ~= 0.49, so its impulse response decays
below float32 resolution within ~48 samples. The IIR is therefore numerically
identical to a 64-tap causal FIR, evaluated on the TensorEngine.

Layout per core: 8 rows x 480000 samples = 128 chunks (8 rows x 16) of 30000
samples, one chunk per partition (all 16 DMA engines engaged). Time is cut
into 120-sample slices; each slice is PE-transposed so within-slice time sits
on partitions, then becomes the stationary operand of a float32r matmul
against banded coefficient matrices [HB | HA]: one matmul per slice writes
that slice's FIR output plus its tail contribution to the next slice,
staggered-accumulated in a single PSUM bank (a full-width start=True carry
matmul [HA | 0] defines every element's first write). The accumulated bank is
y in natural layout -> one copy -> contiguous DMA out. Input DMAs ride the
sync HWDGE ring, output DMAs the scalar ring, 2400 samples/partition per
transfer (1.23 MB), quad-buffered. Measured ~95 us on hardware vs a ~85 us
pure-DMA floor for the 30.7 MB/core of HBM traffic.
"""

import sys

sys.path.insert(0, "/opt/trn_rl_repo")

import numpy as np

import concourse.mybir as mybir
import concourse.tile as tile
from concourse import bacc
from concourse.bass_utils import run_bass_kernel_spmd

f32 = mybir.dt.float32
f32r = mybir.dt.float32r

# ---- problem constants ----------------------------------------------------
SR = 24000
CUTOFF = 8000.0
Q = 0.707

B_FULL, T = 64, 480000
N_CORES = 8
R = B_FULL // N_CORES          # rows per core
NCH = 16                       # chunks per row
P = R * NCH                    # 128 partitions (one chunk each)
L = T // NCH                   # 30000 samples per chunk
LS = 120                       # slice length
NSL = L // LS                  # 250 slices per chunk
S = 4                          # slices per matmul block
G = 5                          # matmul blocks per DMA transfer
D = 64                         # FIR taps kept (h[48] ~ 1e-15 already)
MMW = 256                      # matmul width (>=256 keeps float32r at 1 cyc/col)

MM_DT = f32r                   # conv matmul dtype
TP_DT = f32r                   # transpose dtype


def _filter_mats():
    w0 = 2.0 * np.pi * CUTOFF / SR
    alpha = np.sin(w0) / (2.0 * Q)
    cos_w0 = np.cos(w0)
    b0 = (1.0 - cos_w0) / 2.0
    b1 = 1.0 - cos_w0
    b2 = b0
    a0 = 1.0 + alpha
    a1 = -2.0 * cos_w0
    a2 = 1.0 - alpha
    b0, b1, b2, a1, a2 = (np.float32(b0 / a0), np.float32(b1 / a0),
                          np.float32(b2 / a0), np.float32(a1 / a0),
                          np.float32(a2 / a0))
    # impulse response in float64 using the float32-rounded coefficients
    h = np.zeros(D, dtype=np.float64)
    x1 = x2 = y1 = y2 = 0.0
    for t in range(D):
        x = 1.0 if t == 0 else 0.0
        y = (float(b0) * x + float(b1) * x1 + float(b2) * x2
             - float(a1) * y1 - float(a2) * y2)
        h[t] = y
        x2, x1 = x1, x
        y2, y1 = y1, y
    HB = np.zeros((LS, LS), dtype=np.float32)
    HA = np.zeros((LS, LS), dtype=np.float32)
    for k in range(LS):
        for i in range(LS):
            d = i - k
            if 0 <= d < D:
                HB[k, i] = h[d]
            d2 = LS + i - k
            if 0 < d2 < D:
                HA[k, i] = h[d2]
    pad = MMW - 2 * LS
    Zp = np.zeros((LS, pad), dtype=np.float32)
    Z = np.zeros((LS, LS), dtype=np.float32)
    HBA = np.concatenate([HB, HA, Zp], axis=1)            # [LS, MMW]
    ZHB = np.concatenate([Z, HB, Zp], axis=1)             # [LS, MMW]
    HAZ = np.concatenate(                                  # [LS, 512]
        [HA, np.zeros((LS, 512 - LS), dtype=np.float32)], axis=1)
    # one packed constant block: [128, 2*MMW + 512 + 128]
    def pad128(m):
        return np.concatenate(
            [m, np.zeros((128 - m.shape[0], m.shape[1]), np.float32)], axis=0)
    blk = np.concatenate(
        [pad128(HBA), pad128(ZHB), pad128(HAZ), np.eye(128, dtype=np.float32)],
        axis=1)
    return blk


def _filter_mats_bf16():
    import ml_dtypes
    blk = _filter_mats()          # f32 [128, 2*MMW + 512 + 128]
    HBA = blk[:, 0: MMW][:, : 2 * LS]                 # [128, 240]
    ZHB = blk[:, MMW: 2 * MMW][:, : 2 * LS]
    HAZ = blk[:, 2 * MMW: 2 * MMW + 512][:, : 4 * LS]  # [128, 480]
    ident = blk[:, 2 * MMW + 512:]
    b = np.concatenate([HBA, ZHB, HAZ, ident], axis=1)  # [128, 1088]
    return b.astype(ml_dtypes.bfloat16)


def _build_bf16():
    import ml_dtypes
    bf16 = mybir.dt.bfloat16
    W = 2 * LS                       # matmul width (bf16: 1 cyc/col at any width)
    CONST_np = _filter_mats_bf16()
    nc = bacc.Bacc("TRN2", target_bir_lowering=False)

    wav = nc.dram_tensor("wav", [R, T], f32, kind="ExternalInput")
    out = nc.dram_tensor("out", [R, T], f32, kind="ExternalOutput")
    const_d = nc.inline_tensor(CONST_np, name="constblk16")

    wav_ch = wav[:, :].rearrange("r (c l) -> (r c) l", c=NCH)   # [128, 30000]
    out_ch = out[:, :].rearrange("r (c l) -> (r c) l", c=NCH)

    subs = []
    done = 0
    while done < NSL:
        s = min(S, NSL - done)
        subs.append((done, s))
        done += s
    groups = [subs[i: i + G] for i in range(0, len(subs), G)]

    with tile.TileContext(nc) as tc:
        with (
            tc.tile_pool(name="const", bufs=1) as cpool,
            tc.tile_pool(name="io", bufs=5) as iopool,
            tc.tile_pool(name="work", bufs=6) as wpool,
            tc.tile_pool(name="psum", bufs=4, space="PSUM") as ppool,
        ):
            cblk = cpool.tile([128, 2 * W + 4 * LS + 128], bf16)
            nc.sync.dma_start(cblk[:], const_d[:, :])
            hBA = cblk[:LS, 0: W]
            hZB = cblk[:LS, W: 2 * W]
            hAZ = cblk[:LS, 2 * W: 2 * W + 4 * LS]
            ident = cblk[:, 2 * W + 4 * LS:]

            c0 = cpool.tile([P, LS], bf16)
            nc.gpsimd.memset(c0[:], 0.0)
            for r in range(R):
                nc.gpsimd.dma_start(
                    c0[r * NCH + 1: r * NCH + NCH, :],
                    wav_ch[r * NCH: r * NCH + NCH - 1, L - LS: L],
                )
            pc0 = ppool.tile([LS, P], bf16, tag="pt")
            nc.tensor.transpose(pc0[:], c0[:], ident)
            c0T = cpool.tile([LS, P], bf16)
            nc.scalar.copy(c0T[:], pc0[:, :])

            prev_slab = None
            prev_s = None
            for gi, grp in enumerate(groups):
                gbase = grp[0][0] * LS
                gw = sum(s for (_, s) in grp) * LS

                xin = iopool.tile([P, G * S * LS], bf16, tag="xin")
                nc.gpsimd.dma_start(xin[:, :gw], wav_ch[:, gbase: gbase + gw])
                yout = iopool.tile([P, G * S * LS], f32, tag="yout")

                for (sl0, s) in grp:
                    off = sl0 * LS - gbase
                    w = s * LS

                    pt = ppool.tile([LS, S * P], bf16, tag="pt")
                    for j in range(s):
                        nc.tensor.transpose(
                            pt[:, j * P: (j + 1) * P],
                            xin[:, off + j * LS: off + (j + 1) * LS],
                            ident,
                        )

                    slab = wpool.tile([LS, S * P], bf16, tag="slab")
                    nc.scalar.copy(slab[:, : s * P], pt[:, : s * P])
                    carry = (c0T[:, :] if prev_slab is None
                             else prev_slab[:, (prev_s - 1) * P: prev_s * P])

                    py = ppool.tile([P, S * LS], f32, tag="py")
                    cw = s * LS
                    nc.tensor.matmul(
                        py[:, : cw], carry, hAZ[:, : cw],
                        start=True, stop=False,
                    )
                    for j in range(s):
                        if j < s - 1:
                            o, hmat = j * LS, hBA
                        else:
                            o, hmat = (s - 2) * LS, hZB
                        nc.tensor.matmul(
                            py[:, o: o + W],
                            slab[:, j * P: (j + 1) * P],
                            hmat,
                            start=False, stop=(j == s - 1),
                        )

                    nc.vector.tensor_copy(yout[:, off: off + w], py[:, :w])

                    prev_slab = slab
                    prev_s = s

                nc.scalar.dma_start(out_ch[:, gbase: gbase + gw], yout[:, :gw])

    nc.finalize()
    return nc


BF16_IN = False
BF16_TRANSIT = False


def _build():
    if BF16_IN:
        return _build_bf16()
    CONST_np = _filter_mats()
    nc = bacc.Bacc("TRN2", target_bir_lowering=False)

    wav = nc.dram_tensor("wav", [R, T], f32, kind="ExternalInput")
    out = nc.dram_tensor("out", [R, T], f32, kind="ExternalOutput")
    const_d = nc.inline_tensor(CONST_np, name="constblk")

    wav_ch = wav[:, :].rearrange("r (c l) -> (r c) l", c=NCH)   # [128, 30000]
    out_ch = out[:, :].rearrange("r (c l) -> (r c) l", c=NCH)

    # sub-iterations of up to S slices, grouped G sub-iters per DMA
    subs = []
    done = 0
    while done < NSL:
        s = min(S, NSL - done)
        subs.append((done, s))
        done += s
    groups = [subs[i: i + G] for i in range(0, len(subs), G)]

    with tile.TileContext(nc) as tc:
        with (
            tc.tile_pool(name="const", bufs=1) as cpool,
            tc.tile_pool(name="io", bufs=5) as iopool,
            tc.tile_pool(name="work", bufs=6) as wpool,
            tc.tile_pool(name="psum", bufs=4, space="PSUM") as ppool,
        ):
            cblk = cpool.tile([128, 2 * MMW + 512 + 128], f32)
            nc.sync.dma_start(cblk[:].bitcast(MM_DT), const_d[:, :].bitcast(MM_DT))
            hBA = cblk[:LS, 0: MMW]
            hZB = cblk[:LS, MMW: 2 * MMW]
            hAZ = cblk[:LS, 2 * MMW: 2 * MMW + 512]
            ident = cblk[:, 2 * MMW + 512:]

            # initial carry: the LS samples preceding each chunk (zeros for
            # row-initial chunks), transposed into slab-slice layout.
            c0 = cpool.tile([P, LS], f32)
            nc.gpsimd.memset(c0[:], 0.0)
            for r in range(R):
                nc.gpsimd.dma_start(
                    c0[r * NCH + 1: r * NCH + NCH, :],
                    wav_ch[r * NCH: r * NCH + NCH - 1, L - LS: L],
                )
            pc0 = ppool.tile([LS, P], f32, tag="pt")
            nc.tensor.transpose(pc0[:], c0[:], ident)
            c0T = cpool.tile([LS, P], f32)
            nc.scalar.copy(c0T[:].bitcast(MM_DT), pc0[:, :])

            prev_slab = None
            prev_s = None
            for gi, grp in enumerate(groups):
                gbase = grp[0][0] * LS
                gw = sum(s for (_, s) in grp) * LS

                xin = iopool.tile([P, G * S * LS], f32, tag="xin")
                if BF16_TRANSIT:
                    # SWDGE casts f32->bf16 inside the DMA engines: halves the
                    # SBUF-port traffic of the input stream; DVE upcasts back.
                    xin16 = iopool.tile([P, G * S * LS], mybir.dt.bfloat16,
                                        tag="xin16")
                    nc.gpsimd.dma_start(xin16[:, :gw],
                                        wav_ch[:, gbase: gbase + gw])
                    nc.vector.tensor_copy(xin[:, :gw].bitcast(TP_DT),
                                          xin16[:, :gw])
                else:
                    nc.sync.dma_start(
                        xin[:, :gw].bitcast(TP_DT),
                        wav_ch[:, gbase: gbase + gw].bitcast(TP_DT),
                    )
                yout = iopool.tile([P, G * S * LS], f32, tag="yout")

                for (sl0, s) in grp:
                    off = sl0 * LS - gbase      # sample offset within group
                    w = s * LS

                    # transpose s slices of [P, LS] -> [LS, P] into PSUM
                    pt = ppool.tile([LS, S * P], f32, tag="pt")
                    for j in range(s):
                        nc.tensor.transpose(
                            pt[:, j * P: (j + 1) * P].bitcast(TP_DT),
                            xin[:, off + j * LS: off + (j + 1) * LS].bitcast(TP_DT),
                            ident.bitcast(TP_DT),
                        )

                    slab = wpool.tile([LS, S * P], f32, tag="slab")
                    nc.scalar.copy(slab[:, : s * P].bitcast(MM_DT), pt[:, : s * P])
                    carry = (c0T[:, :] if prev_slab is None
                             else prev_slab[:, (prev_s - 1) * P: prev_s * P])

                    # staggered-accumulation FIR, y in natural layout:
                    #   carry matmul (start=True, [HA|0...]) covers every column
                    #   later matmuls touch; matmul j accumulates [HB|HA|0] at
                    #   j*LS; the last one uses [0|HB|0] shifted back to stay
                    #   inside the bank.
                    py = ppool.tile([P, S * LS + (MMW - 2 * LS)], f32, tag="py")
                    cw = min(512, (s - 2) * LS + MMW)
                    nc.tensor.matmul(
                        py[:, : cw],
                        carry.bitcast(MM_DT),
                        hAZ[:, : cw].bitcast(MM_DT),
                        start=True, stop=False,
                    )
                    for j in range(s):
                        if j < s - 1:
                            o, hmat = j * LS, hBA
                        else:
                            o, hmat = (s - 2) * LS, hZB
                        nc.tensor.matmul(
                            py[:, o: o + MMW],
                            slab[:, j * P: (j + 1) * P].bitcast(MM_DT),
                            hmat.bitcast(MM_DT),
                            start=False, stop=(j == s - 1),
                        )

                    nc.vector.tensor_copy(yout[:, off: off + w], py[:, :w])

                    prev_slab = slab
                    prev_s = s

                nc.scalar.dma_start(out_ch[:, gbase: gbase + gw], yout[:, :gw])

    nc.finalize()
    return nc


_NC_CACHE = None


def _get_nc():
    global _NC_CACHE
    if _NC_CACHE is None:
        _NC_CACHE = _build()
    return _NC_CACHE


def _run(wav_full: np.ndarray, trace: bool = False):
    nc = _get_nc()
    wav_full = np.ascontiguousarray(wav_full, dtype=np.float32)
    in_maps = [
        {"wav": wav_full[i * R: (i + 1) * R]} for i in range(N_CORES)
    ]
    res = run_bass_kernel_spmd(
        nc, in_maps, core_ids=list(range(N_CORES)), trace=trace
    )
    out = np.concatenate([res.results[i]["out"] for i in range(N_CORES)], axis=0)
    return out, res


def kernel(wav: np.ndarray) -> np.ndarray:
    out, _ = _run(np.asarray(wav))
    return out
